# revision 6
# baseline (speedup 1.0000x reference)
"""Chamfer distance kernel for Trainium2 (8 NeuronCores, SPMD).

Problem: xyz1 [4, 8192, 3], xyz2 [4, 8192, 3] (fp32) ->
    scalar = mean_i min_j |x_i - y_j|^2  +  mean_j min_i |x_i - y_j|^2
(means taken over all batches).

Sharding: 8 cores = 4 batches x 2 halves of the N (xyz1-row) dimension.
Core c handles batch c//2, rows [(c%2)*4096, (c%2+1)*4096) of xyz1 and all
8192 rows of xyz2 for that batch.

Per core, the [4096, 8192] squared-distance matrix is produced by the
TensorEngine as a single K=5 fp32 matmul per tile:
    d_ij = x_i . (-2 y_j) + |x_i|^2 * 1 + 1 * |y_j|^2
with lhsT rows [x0, x1, x2, |x|^2, 1] and rhs rows [-2y0, -2y1, -2y2, 1, |y|^2]
(both precomputed on the host in fp32).

Pipeline (raw Bass, one explicit semaphore wait per instruction — the
toolchain rejects instructions carrying more than one sync wait):
  PE:  4 matmuls [128,512] -> one PSUM group [128,2048]; 2 groups ping-pong.
  ACT: copy each PSUM group -> SBUF row buffer S (cast fp32->fp16).
  DVE: per 128-row block:
         col-min accumulate  M2 = min(M2, S)           (fp16 2x mode)
         fused row-min:      tensor_tensor_reduce
             out  = min(S[:, :4096], S[:, 4096:])      (scratch, in place)
             accum= min-reduce(out) -> rowmins[:, j]   (init 3e38)
Host finishes the tiny reductions (min over 128 partitions of M2, min of
the two cores sharing a batch, then the means).

fp16 for the min stages keeps each d to ~5e-4 relative error; the final
means average the (symmetric) rounding noise down to ~1e-5.
"""

import numpy as np

import concourse.bass as bass
from concourse import mybir
from concourse.bass_utils import run_bass_kernel_spmd

# Problem geometry (hardcoded per contest rules).
B = 4
N = 8192
M = 8192
NCORES = 8
HALF = N // 2            # xyz1 rows per core
P = 128                  # partitions
NBLK = HALF // P         # 32 row blocks per core
MM_FREE = 512            # matmul free dim (one PSUM bank of fp32)
PSUM_GROUP = 2048        # psum tensor free dim (4 banks)
NGRP = M // PSUM_GROUP   # 4 psum groups per block row

F32 = mybir.dt.float32
F16 = mybir.dt.float16
MIN = mybir.AluOpType.min

_CACHED_NC = None


def _build_nc():
    from contextlib import ExitStack

    nc = bass.Bass("TRN2", target_bir_lowering=False, debug=False)

    lhsT_d = nc.dram_tensor("lhsT5", [5, HALF], F32, kind="ExternalInput")
    rhs_d = nc.dram_tensor("rhs5", [5, M], F32, kind="ExternalInput")
    rowmins_d = nc.dram_tensor("rowmins", [P, NBLK], F32, kind="ExternalOutput")
    colmin_d = nc.dram_tensor("colmin", [P, M], F16, kind="ExternalOutput")

    with ExitStack() as ctx:
        ec = ctx.enter_context
        lhsT = ec(nc.sbuf_tensor([5, HALF], F32))
        rhs = ec(nc.sbuf_tensor([5, M], F32))
        s0 = ec(nc.sbuf_tensor([P, M], F16))
        s1 = ec(nc.sbuf_tensor([P, M], F16))
        m2 = ec(nc.sbuf_tensor([P, M], F16))
        rowmins = ec(nc.sbuf_tensor([P, NBLK], F32))
        p0 = ec(nc.psum_tensor([P, PSUM_GROUP], F32))
        p1 = ec(nc.psum_tensor([P, PSUM_GROUP], F32))
        dma_sem = ec(nc.semaphore())
        pe_sem = ec(nc.semaphore())
        act_sem = ec(nc.semaphore())
        dve_sem = ec(nc.semaphore())
        block = ec(nc.Block())

        S = [s0, s1]
        PT = [p0, p1]

        @block.sync
        def _(sync):
            sync.dma_start(out=lhsT[:], in_=lhsT_d.ap()).then_inc(dma_sem, 16)
            sync.dma_start(out=rhs[:], in_=rhs_d.ap()).then_inc(dma_sem, 16)
            # wait for the whole DVE program, then ship results
            sync.wait_ge(dve_sem, 2 * NBLK)
            sync.dma_start(out=colmin_d.ap(), in_=m2[:]).then_inc(dma_sem, 16)
            sync.dma_start(out=rowmins_d.ap(), in_=rowmins[:]).then_inc(dma_sem, 16)

        @block.tensor
        def _(tensor):
            tensor.wait_ge(dma_sem, 32)
            for j in range(NBLK):
                for q in range(NGRP):
                    g = NGRP * j + q
                    if g >= 2:
                        # psum buffer g%2 must have been drained by ACT (group g-2)
                        tensor.wait_ge(act_sem, g - 1)
                    pt = PT[g % 2]
                    mm = None
                    for t in range(PSUM_GROUP // MM_FREE):
                        c = (q * (PSUM_GROUP // MM_FREE) + t) * MM_FREE
                        mm = nc.tensor.matmul(
                            pt[:, t * MM_FREE:(t + 1) * MM_FREE],
                            lhsT[:, j * P:(j + 1) * P],
                            rhs[:, c:c + MM_FREE],
                            start=True,
                            stop=True,
                        )
                    mm.then_inc(pe_sem, 1)  # MMs retire in order; last inc suffices

        @block.scalar
        def _(scalar):
            for j in range(NBLK):
                for q in range(NGRP):
                    g = NGRP * j + q
                    if q == 0 and j >= 2:
                        # S buffer j%2 must have been fully consumed by DVE
                        # (ttr of block j-2 is its last reader/writer)
                        scalar.wait_ge(dve_sem, 2 * (j - 1))
                    scalar.wait_ge(pe_sem, g + 1)
                    nc.scalar.copy(
                        out=S[j % 2][:, q * PSUM_GROUP:(q + 1) * PSUM_GROUP],
                        in_=PT[g % 2][:],
                    ).then_inc(act_sem, 1)

        @block.vector
        def _(vector):
            nc.vector.memset(m2[:], 60000.0)
            for j in range(NBLK):
                vector.wait_ge(act_sem, NGRP * (j + 1))
                s = S[j % 2]
                nc.vector.tensor_tensor(
                    out=m2[:], in0=m2[:], in1=s[:], op=MIN
                ).then_inc(dve_sem, 1)
                # row-min: in-place fold tree (fp16 2x mode) down to 512,
                # then one 1x reduce into rowmins[:, j]
                w = M // 2
                while w >= 512:
                    nc.vector.tensor_tensor(
                        out=s[:, :w], in0=s[:, :w], in1=s[:, w:2 * w], op=MIN
                    )
                    w //= 2
                nc.vector.tensor_reduce(
                    out=rowmins[:, j:j + 1],
                    in_=s[:, : 2 * w],
                    axis=mybir.AxisListType.X,
                    op=MIN,
                ).then_inc(dve_sem, 1)

    return nc


def _get_nc():
    global _CACHED_NC
    if _CACHED_NC is None:
        _CACHED_NC = _build_nc()
    return _CACHED_NC


def _make_in_maps(xyz1, xyz2):
    xyz1 = np.asarray(xyz1, dtype=np.float32)
    xyz2 = np.asarray(xyz2, dtype=np.float32)
    in_maps = []
    for c in range(NCORES):
        b, h = divmod(c, 2)
        x = xyz1[b, h * HALF:(h + 1) * HALF]      # [4096, 3]
        y = xyz2[b]                               # [8192, 3]
        lhsT5 = np.empty((5, HALF), np.float32)
        lhsT5[0:3] = x.T
        lhsT5[3] = (x.astype(np.float64) ** 2).sum(1)
        lhsT5[4] = 1.0
        rhs5 = np.empty((5, M), np.float32)
        rhs5[0:3] = -2.0 * y.T
        rhs5[3] = 1.0
        rhs5[4] = (y.astype(np.float64) ** 2).sum(1)
        in_maps.append({"lhsT5": lhsT5, "rhs5": rhs5})
    return in_maps


def _combine(results):
    # mean over all row-min (dist1) values: every core contributes 4096 rows
    d1 = np.stack([np.asarray(r["rowmins"], np.float64) for r in results])
    # per-core col-min partials [128, 8192] -> [8192], then min across the
    # two cores sharing each batch
    cm = np.stack(
        [np.asarray(r["colmin"], np.float64).min(axis=0) for r in results]
    )  # [8, 8192]
    dist2 = np.minimum(cm[0::2], cm[1::2])  # [4, 8192]
    return np.float32(d1.mean() + dist2.mean())


def _run(xyz1, xyz2, trace=False):
    nc = _get_nc()
    in_maps = _make_in_maps(xyz1, xyz2)
    res = run_bass_kernel_spmd(nc, in_maps, list(range(NCORES)), trace=trace)
    return _combine(res.results), res


def kernel(xyz1, xyz2):
    out, _ = _run(xyz1, xyz2, trace=False)
    return out


# revision 11
# speedup vs baseline: 2.8145x; 2.8145x over previous
"""Chamfer distance kernel for Trainium2 (8 NeuronCores, SPMD).

Problem: xyz1 [4, 8192, 3], xyz2 [4, 8192, 3] (fp32) ->
    scalar = mean_i min_j |x_i - y_j|^2  +  mean_j min_i |x_i - y_j|^2
(means taken over all batches).

Sharding: 8 cores = 4 batches x 2 halves of the N (xyz1-row) dimension.
Core c handles batch c//2, rows [(c%2)*4096, (c%2+1)*4096) of xyz1 and all
8192 rows of xyz2 for that batch.

Per core, the [4096, 8192] squared-distance matrix is produced by the
TensorEngine as a single K=13 fp16 matmul per tile:
    d_ij = x_i . (-2 y_j) + |x_i|^2 * 1 + 1 * |y_j|^2
Every fp32 operand is split into fp16 hi+lo halves (a = ah + al with
ah = fp16(a)); each x.t coordinate product uses the three dominant terms
xh*th + xh*tl + xl*th (the dropped xl*tl is ~2^-22 relative), and the
norm rows are carried as hi+lo against a row of ones.  This runs the PE
at full 16-bit stream rate — fp32 matmuls on TRN2 are split by the
compiler into two half-rate passes (measured 4x slower end to end).

Pipeline (raw Bass, one explicit semaphore wait per instruction — the
toolchain rejects instructions carrying more than one sync wait):
  PE:  4 matmuls [128,512] -> one PSUM group [128,2048]; 2 groups ping-pong.
  ACT: copy each PSUM group -> SBUF row buffer S (cast fp32->fp16).
  DVE: per 128-row block:
         col-min accumulate  M2 = min(M2, S)           (fp16 2x mode)
         fused row-min:      tensor_tensor_reduce
             out  = min(S[:, :4096], S[:, 4096:])      (scratch, in place)
             accum= min-reduce(out) -> rowmins[:, j]   (init 3e38)
Host finishes the tiny reductions (min over 128 partitions of M2, min of
the two cores sharing a batch, then the means).

fp16 for the min stages keeps each d to ~5e-4 relative error; the final
means average the (symmetric) rounding noise down to ~1e-5.
"""

import numpy as np

import concourse.bass as bass
from concourse import mybir
from concourse.bass_utils import run_bass_kernel_spmd

# Problem geometry (hardcoded per contest rules).
B = 4
N = 8192
M = 8192
NCORES = 8
HALF = N // 2            # xyz1 rows per core
P = 128                  # partitions
NBLK = HALF // P         # 32 row blocks per core
MM_FREE = 512            # matmul free dim (one PSUM bank of fp32)
PSUM_GROUP = 2048        # psum tensor free dim (4 banks)
NGRP = M // PSUM_GROUP   # 4 psum groups per block row

KDIM = 13                # 3 coords x 3 split-product terms + 2x2 norm rows

F32 = mybir.dt.float32
F16 = mybir.dt.float16
MIN = mybir.AluOpType.min

_CACHED_NC = None


def _build_nc():
    from contextlib import ExitStack

    nc = bass.Bass("TRN2", target_bir_lowering=False, debug=False)

    lhsT_d = nc.dram_tensor("lhsT5", [KDIM, HALF], F16, kind="ExternalInput")
    rhs_d = nc.dram_tensor("rhs5", [KDIM, M], F16, kind="ExternalInput")
    rowmins_d = nc.dram_tensor("rowmins", [P, NBLK], F32, kind="ExternalOutput")
    colmin_d = nc.dram_tensor("colmin", [P, M], F16, kind="ExternalOutput")

    with ExitStack() as ctx:
        ec = ctx.enter_context
        lhsT = ec(nc.sbuf_tensor([KDIM, HALF], F16))
        rhs = ec(nc.sbuf_tensor([KDIM, M], F16))
        s0 = ec(nc.sbuf_tensor([P, M], F16))
        s1 = ec(nc.sbuf_tensor([P, M], F16))
        m2 = ec(nc.sbuf_tensor([P, M], F16))
        rowmins = ec(nc.sbuf_tensor([P, NBLK], F32))
        p0 = ec(nc.psum_tensor([P, PSUM_GROUP], F32))
        p1 = ec(nc.psum_tensor([P, PSUM_GROUP], F32))
        dma_sem = ec(nc.semaphore())
        pe_sem = ec(nc.semaphore())
        act_sem = ec(nc.semaphore())
        dve_sem = ec(nc.semaphore())
        block = ec(nc.Block())

        S = [s0, s1]
        PT = [p0, p1]

        @block.sync
        def _(sync):
            sync.dma_start(out=lhsT[:], in_=lhsT_d.ap()).then_inc(dma_sem, 16)
            sync.dma_start(out=rhs[:], in_=rhs_d.ap()).then_inc(dma_sem, 16)
            # wait for the whole DVE program, then ship results
            sync.wait_ge(dve_sem, 2 * NBLK)
            sync.dma_start(out=colmin_d.ap(), in_=m2[:]).then_inc(dma_sem, 16)
            sync.dma_start(out=rowmins_d.ap(), in_=rowmins[:]).then_inc(dma_sem, 16)

        @block.tensor
        def _(tensor):
            tensor.wait_ge(dma_sem, 32)
            for j in range(NBLK):
                for q in range(NGRP):
                    g = NGRP * j + q
                    if g >= 2:
                        # psum buffer g%2 must have been drained by ACT (group g-2)
                        tensor.wait_ge(act_sem, g - 1)
                    pt = PT[g % 2]
                    mm = None
                    for t in range(PSUM_GROUP // MM_FREE):
                        c = (q * (PSUM_GROUP // MM_FREE) + t) * MM_FREE
                        mm = nc.tensor.matmul(
                            pt[:, t * MM_FREE:(t + 1) * MM_FREE],
                            lhsT[:, j * P:(j + 1) * P],
                            rhs[:, c:c + MM_FREE],
                            start=True,
                            stop=True,
                        )
                    mm.then_inc(pe_sem, 1)  # MMs retire in order; last inc suffices

        @block.scalar
        def _(scalar):
            for j in range(NBLK):
                for q in range(NGRP):
                    g = NGRP * j + q
                    if q == 0 and j >= 2:
                        # S buffer j%2 must have been fully consumed by DVE
                        # (ttr of block j-2 is its last reader/writer)
                        scalar.wait_ge(dve_sem, 2 * (j - 1))
                    scalar.wait_ge(pe_sem, g + 1)
                    nc.scalar.copy(
                        out=S[j % 2][:, q * PSUM_GROUP:(q + 1) * PSUM_GROUP],
                        in_=PT[g % 2][:],
                    ).then_inc(act_sem, 1)

        @block.vector
        def _(vector):
            nc.vector.memset(m2[:], 60000.0)
            for j in range(NBLK):
                vector.wait_ge(act_sem, NGRP * (j + 1))
                s = S[j % 2]
                nc.vector.tensor_tensor(
                    out=m2[:], in0=m2[:], in1=s[:], op=MIN
                ).then_inc(dve_sem, 1)
                # row-min: in-place fold tree (fp16 2x mode) down to 512,
                # then one 1x reduce into rowmins[:, j]
                w = M // 2
                while w >= 512:
                    nc.vector.tensor_tensor(
                        out=s[:, :w], in0=s[:, :w], in1=s[:, w:2 * w], op=MIN
                    )
                    w //= 2
                nc.vector.tensor_reduce(
                    out=rowmins[:, j:j + 1],
                    in_=s[:, : 2 * w],
                    axis=mybir.AxisListType.X,
                    op=MIN,
                ).then_inc(dve_sem, 1)

    return nc


def _get_nc():
    global _CACHED_NC
    if _CACHED_NC is None:
        _CACHED_NC = _build_nc()
    return _CACHED_NC


def _split16(a):
    """fp32/fp64 -> (hi, lo) fp16 with hi + lo ~= a to ~2^-22."""
    hi = a.astype(np.float16)
    lo = (a - hi.astype(np.float64)).astype(np.float16)
    return hi, lo


def _make_in_maps(xyz1, xyz2):
    xyz1 = np.asarray(xyz1, dtype=np.float32)
    xyz2 = np.asarray(xyz2, dtype=np.float32)
    in_maps = []
    for c in range(NCORES):
        b, h = divmod(c, 2)
        x = xyz1[b, h * HALF:(h + 1) * HALF].astype(np.float64)  # [4096, 3]
        t = -2.0 * xyz2[b].astype(np.float64)                    # [8192, 3]
        xh, xl = _split16(x)
        th, tl = _split16(t)
        nxh, nxl = _split16((x ** 2).sum(1))
        # |y|^2 = |t/2|^2 = (t/2)**2 summed
        nyh, nyl = _split16(((t / 2.0) ** 2).sum(1))

        lhsT5 = np.empty((KDIM, HALF), np.float16)
        rhs5 = np.empty((KDIM, M), np.float16)
        for ci in range(3):
            lhsT5[3 * ci + 0] = xh[:, ci]
            lhsT5[3 * ci + 1] = xh[:, ci]
            lhsT5[3 * ci + 2] = xl[:, ci]
            rhs5[3 * ci + 0] = th[:, ci]
            rhs5[3 * ci + 1] = tl[:, ci]
            rhs5[3 * ci + 2] = th[:, ci]
        lhsT5[9] = nxh
        lhsT5[10] = nxl
        lhsT5[11] = 1.0
        lhsT5[12] = 1.0
        rhs5[9] = 1.0
        rhs5[10] = 1.0
        rhs5[11] = nyh
        rhs5[12] = nyl
        in_maps.append({"lhsT5": lhsT5, "rhs5": rhs5})
    return in_maps


def _combine(results):
    # mean over all row-min (dist1) values: every core contributes 4096 rows
    d1 = np.stack([np.asarray(r["rowmins"], np.float64) for r in results])
    # per-core col-min partials [128, 8192] -> [8192], then min across the
    # two cores sharing each batch
    cm = np.stack(
        [np.asarray(r["colmin"], np.float64).min(axis=0) for r in results]
    )  # [8, 8192]
    dist2 = np.minimum(cm[0::2], cm[1::2])  # [4, 8192]
    return np.float32(d1.mean() + dist2.mean())


def _run(xyz1, xyz2, trace=False):
    nc = _get_nc()
    in_maps = _make_in_maps(xyz1, xyz2)
    res = run_bass_kernel_spmd(nc, in_maps, list(range(NCORES)), trace=trace)
    return _combine(res.results), res


def kernel(xyz1, xyz2):
    out, _ = _run(xyz1, xyz2, trace=False)
    return out


# revision 14
# speedup vs baseline: 3.1457x; 1.1177x over previous
"""Chamfer distance kernel for Trainium2 (8 NeuronCores, SPMD).

Problem: xyz1 [4, 8192, 3], xyz2 [4, 8192, 3] (fp32) ->
    scalar = mean_i min_j |x_i - y_j|^2  +  mean_j min_i |x_i - y_j|^2
(means taken over all batches).

Sharding: 8 cores = 4 batches x 2 halves of the N (xyz1-row) dimension.
Core c handles batch c//2, rows [(c%2)*4096, (c%2+1)*4096) of xyz1 and all
8192 rows of xyz2 for that batch.

Per core, the [4096, 8192] squared-distance matrix is produced by the
TensorEngine as a single K=13 fp16 matmul per tile:
    d_ij = x_i . (-2 y_j) + |x_i|^2 * 1 + 1 * |y_j|^2
Every fp32 operand is split into fp16 hi+lo halves (a = ah + al with
ah = fp16(a)); each x.t coordinate product uses the three dominant terms
xh*th + xh*tl + xl*th (the dropped xl*tl is ~2^-22 relative), and the
norm rows are carried as hi+lo against a row of ones.  This runs the PE
at full 16-bit stream rate — fp32 matmuls on TRN2 are split by the
compiler into two half-rate passes (measured 4x slower end to end).

Pipeline (raw Bass, one explicit semaphore wait per instruction — the
toolchain rejects instructions carrying more than one sync wait), blocks
processed in PAIRS (even j=2k, odd j=2k+1):
  PE:   4 matmuls [128,512] -> one PSUM group [128,2048]; 2 groups ping-pong.
  ACT:  copy PSUM -> SBUF fp16: even block straight into ship buffer U_k,
        odd block into S_tmp.
  DVE:  row-min of the even block: fold tree U_k -> W scratch -> rowmins;
        col-min pair fold:  U_k = min(U_k, S_tmp)   (fp16 2x, one op/pair);
        row-min of the odd block: fold tree in-place in S_tmp -> rowmins.
  DMA:  ship U_k [128, 8192] fp16 to DRAM (overlapped; DMA is otherwise idle).
Host combines: per-core col-min partial = min over the 16 shipped pair-mins
and their 128 partitions; dist2 = min of the two cores per batch; means in
fp64.

fp16 for the min stages keeps each d to ~5e-4 relative error; the final
means average the (symmetric) rounding noise down to ~1e-5.
"""

import numpy as np

import concourse.bass as bass
from concourse import mybir
from concourse.bass_utils import run_bass_kernel_spmd

# Problem geometry (hardcoded per contest rules).
B = 4
N = 8192
M = 8192
NCORES = 8
HALF = N // 2            # xyz1 rows per core
P = 128                  # partitions
NBLK = HALF // P         # 32 row blocks per core
NPAIR = NBLK // 2        # 16 block pairs -> 16 shipped col-min buffers
MM_FREE = 512            # matmul free dim (one PSUM bank of fp32)
PSUM_GROUP = 2048        # psum tensor free dim (4 banks)
NGRP = M // PSUM_GROUP   # 4 psum groups per block row
KDIM = 13                # 3 coords x 3 split-product terms + 2x2 norm rows

F32 = mybir.dt.float32
F16 = mybir.dt.float16
MIN = mybir.AluOpType.min

NUBUF = 3                # ship-buffer ring (ACT write / DVE min / DMA out)

_CACHED_NC = None


def _build_nc():
    from contextlib import ExitStack

    nc = bass.Bass("TRN2", target_bir_lowering=False, debug=False)

    lhsT_d = nc.dram_tensor("lhsT5", [KDIM, HALF], F16, kind="ExternalInput")
    rhs_d = nc.dram_tensor("rhs5", [KDIM, M], F16, kind="ExternalInput")
    rowmins_d = nc.dram_tensor("rowmins", [P, NBLK], F32, kind="ExternalOutput")
    colmin_d = nc.dram_tensor("colmin", [NPAIR, P, M], F16, kind="ExternalOutput")

    with ExitStack() as ctx:
        ec = ctx.enter_context
        lhsT = ec(nc.sbuf_tensor([KDIM, HALF], F16))
        rhs = ec(nc.sbuf_tensor([KDIM, M], F16))
        u_bufs = [
            ec(nc.sbuf_tensor(f"u{i}", [P, M], F16)) for i in range(NUBUF)
        ]
        s_tmp = [
            ec(nc.sbuf_tensor(f"s{i}", [P, M], F16)) for i in range(2)
        ]
        w = ec(nc.sbuf_tensor([P, M // 2], F16))
        rowmins = ec(nc.sbuf_tensor([P, NBLK], F32))
        p0 = ec(nc.psum_tensor([P, PSUM_GROUP], F32))
        p1 = ec(nc.psum_tensor([P, PSUM_GROUP], F32))
        dma_sem = ec(nc.semaphore())
        pe_sem = ec(nc.semaphore())
        act_sem = ec(nc.semaphore())
        dve_sem = ec(nc.semaphore())
        out_sem = ec(nc.semaphore())
        block = ec(nc.Block())

        PT = [p0, p1]

        # per-pair semaphore targets:
        #  act_sem: +1 per PSUM-group copy -> 2*NGRP per pair
        #  dve_sem: +1 after pair colmin (U_k final), +1 after odd rowmin
        #  out_sem: +16 per shipped U_k

        @block.sync
        def _(sync):
            sync.dma_start(out=lhsT[:], in_=lhsT_d.ap()).then_inc(dma_sem, 16)
            sync.dma_start(out=rhs[:], in_=rhs_d.ap()).then_inc(dma_sem, 16)
            for k in range(NPAIR):
                sync.wait_ge(dve_sem, 2 * k + 1)
                sync.dma_start(
                    out=colmin_d.ap()[k], in_=u_bufs[k % NUBUF][:]
                ).then_inc(out_sem, 16)
            sync.wait_ge(dve_sem, 2 * NPAIR)
            sync.dma_start(out=rowmins_d.ap(), in_=rowmins[:]).then_inc(dma_sem, 16)

        @block.tensor
        def _(tensor):
            tensor.wait_ge(dma_sem, 32)
            for j in range(NBLK):
                for q in range(NGRP):
                    g = NGRP * j + q
                    if g >= 2:
                        # psum buffer g%2 must have been drained by ACT (group g-2)
                        tensor.wait_ge(act_sem, g - 1)
                    pt = PT[g % 2]
                    mm = None
                    for t in range(PSUM_GROUP // MM_FREE):
                        c = (q * (PSUM_GROUP // MM_FREE) + t) * MM_FREE
                        mm = nc.tensor.matmul(
                            pt[:, t * MM_FREE:(t + 1) * MM_FREE],
                            lhsT[:, j * P:(j + 1) * P],
                            rhs[:, c:c + MM_FREE],
                            start=True,
                            stop=True,
                        )
                    mm.then_inc(pe_sem, 1)  # MMs retire in order; last inc suffices

        @block.scalar
        def _(scalar):
            for j in range(NBLK):
                k, odd = divmod(j, 2)
                if odd:
                    dst = s_tmp[k % 2]
                    # S_tmp k%2 fully consumed by DVE after pair k-2's odd rowmin
                    if k >= 2:
                        scalar.wait_ge(dve_sem, 2 * (k - 1))
                else:
                    dst = u_bufs[k % NUBUF]
                    # U ring slot free once pair k-NUBUF was shipped
                    if k >= NUBUF:
                        scalar.wait_ge(out_sem, 16 * (k - NUBUF + 1))
                for q in range(NGRP):
                    g = NGRP * j + q
                    scalar.wait_ge(pe_sem, g + 1)
                    nc.scalar.copy(
                        out=dst[:, q * PSUM_GROUP:(q + 1) * PSUM_GROUP],
                        in_=PT[g % 2][:],
                    ).then_inc(act_sem, 1)

        def rowmin_chain(vector, src, scratch, j):
            """Fold tree: min over the M columns of src -> rowmins[:, j].
            First fold reads src (non-destructively) into scratch; the rest
            fold scratch in place.  fp16 2x mode throughout, final 1x reduce
            at width 512."""
            nc.vector.tensor_tensor(
                out=scratch[:, : M // 2], in0=src[:, : M // 2],
                in1=src[:, M // 2:], op=MIN,
            )
            ww = M // 4
            while ww >= 512:
                nc.vector.tensor_tensor(
                    out=scratch[:, :ww], in0=scratch[:, :ww],
                    in1=scratch[:, ww:2 * ww], op=MIN,
                )
                ww //= 2
            return nc.vector.tensor_reduce(
                out=rowmins[:, j:j + 1], in_=scratch[:, : 2 * ww],
                axis=mybir.AxisListType.X, op=MIN,
            )

        @block.vector
        def _(vector):
            for k in range(NPAIR):
                u = u_bufs[k % NUBUF]
                s = s_tmp[k % 2]
                # even block landed in U
                vector.wait_ge(act_sem, (2 * k + 1) * NGRP)
                rowmin_chain(vector, u, w, 2 * k)
                # odd block landed in S_tmp
                vector.wait_ge(act_sem, (2 * k + 2) * NGRP)
                nc.vector.tensor_tensor(
                    out=u[:], in0=u[:], in1=s[:], op=MIN
                ).then_inc(dve_sem, 1)
                rowmin_chain(vector, s, s, 2 * k + 1).then_inc(dve_sem, 1)

    return nc


def _get_nc():
    global _CACHED_NC
    if _CACHED_NC is None:
        _CACHED_NC = _build_nc()
    return _CACHED_NC


def _split16(a):
    """fp32/fp64 -> (hi, lo) fp16 with hi + lo ~= a to ~2^-22."""
    hi = a.astype(np.float16)
    lo = (a - hi.astype(np.float64)).astype(np.float16)
    return hi, lo


def _make_in_maps(xyz1, xyz2):
    xyz1 = np.asarray(xyz1, dtype=np.float32)
    xyz2 = np.asarray(xyz2, dtype=np.float32)
    in_maps = []
    for c in range(NCORES):
        b, h = divmod(c, 2)
        x = xyz1[b, h * HALF:(h + 1) * HALF].astype(np.float64)  # [4096, 3]
        t = -2.0 * xyz2[b].astype(np.float64)                    # [8192, 3]
        xh, xl = _split16(x)
        th, tl = _split16(t)
        nxh, nxl = _split16((x ** 2).sum(1))
        nyh, nyl = _split16(((t / 2.0) ** 2).sum(1))

        lhsT5 = np.empty((KDIM, HALF), np.float16)
        rhs5 = np.empty((KDIM, M), np.float16)
        for ci in range(3):
            lhsT5[3 * ci + 0] = xh[:, ci]
            lhsT5[3 * ci + 1] = xh[:, ci]
            lhsT5[3 * ci + 2] = xl[:, ci]
            rhs5[3 * ci + 0] = th[:, ci]
            rhs5[3 * ci + 1] = tl[:, ci]
            rhs5[3 * ci + 2] = th[:, ci]
        lhsT5[9] = nxh
        lhsT5[10] = nxl
        lhsT5[11] = 1.0
        lhsT5[12] = 1.0
        rhs5[9] = 1.0
        rhs5[10] = 1.0
        rhs5[11] = nyh
        rhs5[12] = nyl
        in_maps.append({"lhsT5": lhsT5, "rhs5": rhs5})
    return in_maps


def _combine(results):
    # mean over all row-min (dist1) values: every core contributes 4096 rows
    d1 = np.stack([np.asarray(r["rowmins"], np.float64) for r in results])
    # per-core col-min partials [16, 128, 8192]: min over pair-bufs and
    # partitions, then min across the two cores sharing each batch
    cm = np.stack(
        [
            np.asarray(r["colmin"]).astype(np.float32).min(axis=(0, 1))
            for r in results
        ]
    )  # [8, 8192]
    dist2 = np.minimum(cm[0::2], cm[1::2]).astype(np.float64)  # [4, 8192]
    return np.float32(d1.mean() + dist2.mean())


def _run(xyz1, xyz2, trace=False):
    nc = _get_nc()
    in_maps = _make_in_maps(xyz1, xyz2)
    res = run_bass_kernel_spmd(nc, in_maps, list(range(NCORES)), trace=trace)
    return _combine(res.results), res


def kernel(xyz1, xyz2):
    out, _ = _run(xyz1, xyz2, trace=False)
    return out
